# revision 33
# baseline (speedup 1.0000x reference)
"""Distributed multi-head attention kernel for one TRN2 chip (8 NeuronCores).

Problem: B=4, S=2048, D=1024, H=16, Dh=64 fp32 attention
    q,k,v = x@W* + b*  (per head)  ->  softmax(q k^T / sqrt(Dh)) v  -> @Wo + bo

Sharding (per the hint): data-parallel over B (4) x tensor-parallel over
head-halves (2) = 8 cores.  Core c = 2*b + hg handles batch b and heads
[8*hg, 8*hg+8) i.e. d_model slice [512*hg, 512*hg+512).  Each core produces
a partial output [2048, 1024] (its 8 heads' contribution through Wo); the
host sums the two partials per batch and adds bo (the unshard step).

Per-core pipeline:
  - features-on-partitions: Q^T/K^T [dc, S] from the QKV matmuls; scores^T
    tiles land [k_seq, q_seq] with k on partitions.
  - softmax: exp() unnormalized on the Act engine; row-sums come from
    ones-columns appended to V (free: matmul cost is free-dim-bound).
  - normalization (1/rowsum) via reciprocal_approx_fast (custom DVE op,
    ~5x faster than InstReciprocal).  NOTE: custom-DVE ops silently
    no-op on partition-offset APs — always issue them on [0:128].
  - schedule: Q/K m0-n0 projections + first scores go FIRST so exp starts
    ~15us in; V projections fill block 0; remaining Q/K m-groups spread
    over blocks 1-12 (each m-group just before its consuming pair) and
    the output projection over blocks 13-15, sized so each step's PE
    work stays near the 2.2us of exp it feeds.  Final 2MB of output DMA
    fans out over 3 queues (Act's queue is idle by then).

Compute dtypes: bf16 matmul operands, fp32 PSUM accumulate, bf16 output
partials (host sums in fp32).  Measured rel-err ~2.7e-3 (gate 2e-2).
fp8 DoubleRow/SwInterleave AV was tried and REVERTED: on real TRN2 those
matmuls run at ~2x the duration of bf16 (no throughput win, cost model
disagrees with HW) and plain-DR needs no layout change but wins nothing.
"""

import sys

sys.path.insert(0, "/opt/trn_rl_repo")

import numpy as np
import ml_dtypes

from contextlib import ExitStack

import concourse.bass as bass
import concourse.tile as tile
from concourse import bacc, mybir
from concourse.bass_utils import run_bass_kernel_spmd

BF16 = mybir.dt.bfloat16
F32 = mybir.dt.float32
FP8 = mybir.dt.float8e4
AF = mybir.ActivationFunctionType
DR = mybir.MatmulPerfMode.DoubleRow


def _install_ntff_hook():
    """Provide antenv.axon_hooks (missing in this image) so that
    run_bass_kernel_spmd(trace=True) can capture NTFF profiles via the
    axon PJRT .so's C ABI."""
    import types, ctypes, contextlib

    if "antenv.axon_hooks" in sys.modules:
        return
    so_path = "/opt/axon/libaxon_pjrt.so"
    mod = types.ModuleType("antenv.axon_hooks")
    _state = {"hook": None}

    def set_axon_ntff_profile_hook(h):
        _state["hook"] = h

    def get_axon_ntff_profile_hook():
        return _state["hook"]

    mod.set_axon_ntff_profile_hook = set_axon_ntff_profile_hook
    mod.get_axon_ntff_profile_hook = get_axon_ntff_profile_hook
    sys.modules["antenv.axon_hooks"] = mod
    import antenv

    antenv.axon_hooks = mod

    try:
        lib = ctypes.CDLL(so_path)
    except OSError:
        return
    if not hasattr(lib, "axon_start_nrt_profile"):
        return
    lib.axon_start_nrt_profile.argtypes = [
        ctypes.POINTER(ctypes.c_int64),
        ctypes.c_size_t,
    ]
    lib.axon_start_nrt_profile.restype = ctypes.c_int64
    lib.axon_stop_nrt_profile.argtypes = [ctypes.c_char_p]
    lib.axon_stop_nrt_profile.restype = ctypes.c_int64

    @contextlib.contextmanager
    def _hook(output_dir, device_ids):
        import jax

        jax.devices()
        if device_ids:
            ids = (ctypes.c_int64 * len(device_ids))(*device_ids)
            rc = lib.axon_start_nrt_profile(ids, len(device_ids))
        else:
            rc = lib.axon_start_nrt_profile(None, 0)
        if rc != 0:
            raise RuntimeError(f"axon_start_nrt_profile rc={rc}")
        try:
            yield
        finally:
            n = lib.axon_stop_nrt_profile(str(output_dir).encode())
            print(f"profile: {n} file(s) written to {output_dir}",
                  file=sys.stderr)

    set_axon_ntff_profile_hook(_hook)


_install_ntff_hook()

D = 1024          # d_model
DC = 512          # per-core d slice (8 heads)
H_CORE = 8        # heads per core
DH = 64           # head dim
NPAIRS = 4        # head pairs per core


def build_graph(S=2048):
    """Build the per-core Bass graph (same graph on all 8 cores)."""
    nc = bacc.Bacc(
        "TRN2",
        target_bir_lowering=False,
        debug=False,
        enable_asserts=False,
        num_devices=8,
    )

    ST = S // 128       # 128-seq tiles (16)
    T2 = ST // 2        # 256-seq k-pair tiles (8)
    QT_ = S // 512      # 512-seq q blocks (4)

    xT = nc.dram_tensor("xT", [D, S], BF16, kind="ExternalInput").ap()
    # wqm/wkm are m-major and pre-transposed to the SBUF tile layout
    # [m-group, p, kt*128+c] so one contiguous DMA brings the whole
    # m-slice needed by a projection group.
    wqm = nc.dram_tensor("wqm", [4, 128, D], BF16, kind="ExternalInput").ap()
    wkm = nc.dram_tensor("wkm", [4, 128, D], BF16, kind="ExternalInput").ap()
    wv = nc.dram_tensor("wv", [D, DC], BF16, kind="ExternalInput").ap()
    wo = nc.dram_tensor("wo", [DC, D], BF16, kind="ExternalInput").ap()
    bq = nc.dram_tensor("bq", [DC, 1], F32, kind="ExternalInput").ap()
    bk = nc.dram_tensor("bk", [DC, 1], F32, kind="ExternalInput").ap()
    bvb = nc.dram_tensor("bvb", [128, DC], BF16, kind="ExternalInput").ap()
    # bf16 output: halves the 8MB/core output DMA; the host sums the two
    # core partials in fp32.  Costs ~2e-3 extra rel-err, well under gate.
    out = nc.dram_tensor("out", [S, D], BF16, kind="ExternalOutput").ap()

    with tile.TileContext(nc) as tc, ExitStack() as ctx:
        # ---- persistent pools --------------------------------------------
        qt_pool = ctx.enter_context(tc.tile_pool(name="qt", bufs=4))
        kt_pool = ctx.enter_context(tc.tile_pool(name="kt", bufs=4))
        vaug_pool = ctx.enter_context(tc.tile_pool(name="vaug", bufs=ST))
        ctx_pool = ctx.enter_context(tc.tile_pool(name="ctxT", bufs=4))
        const_pool = ctx.enter_context(tc.tile_pool(name="consts", bufs=1))
        wo_pool = ctx.enter_context(tc.tile_pool(name="wo", bufs=4))
        xt_pool = ctx.enter_context(tc.tile_pool(name="xt", bufs=8))
        wqk_pool = ctx.enter_context(tc.tile_pool(name="wqk", bufs=8))
        wv_pool = ctx.enter_context(tc.tile_pool(name="wv", bufs=8))

        qt_tiles = [qt_pool.tile([128, S], BF16, tag="qt", name=f"qt{i}")
                    for i in range(4)]
        kt_tiles = [kt_pool.tile([128, S], BF16, tag="kt", name=f"ktt{i}")
                    for i in range(4)]
        # vaug[st]: [128 kpos, 8 heads x (64 V | 64 ones)] bf16
        vaug_tiles = [vaug_pool.tile([128, H_CORE * 128], BF16, tag="vaug",
                                     name=f"vaug{i}") for i in range(ST)]
        ctx_tiles = [ctx_pool.tile([128, S], BF16, tag="ctxT", name=f"ctxT{i}")
                     for i in range(4)]
        xt_tiles = [xt_pool.tile([128, S], BF16, tag="xt", name=f"xtt{i}")
                    for i in range(8)]
        wqm_tiles = [wqk_pool.tile([128, D], BF16, tag="wqk", name=f"wqm{m}")
                     for m in range(4)]
        wkm_tiles = [wqk_pool.tile([128, D], BF16, tag="wqk", name=f"wkm{m}")
                     for m in range(4)]
        wv_tiles = [wv_pool.tile([128, DC], BF16, tag="wv", name=f"wvt{i}")
                    for i in range(8)]
        wo_tiles = [wo_pool.tile([128, D], BF16, tag="wo", name=f"wot{i}")
                    for i in range(4)]

        bvb_sb = const_pool.tile([128, DC], BF16, tag="bvb")
        bq_sb = const_pool.tile([128, DC // 128], F32, tag="bq")
        bk_sb = const_pool.tile([128, DC // 128], F32, tag="bk")

        # ---- DMA plan: 4 queues, critical prefix first -------------------
        # prefix: xt cols 0:512 (all 8 k-tiles) + wkm[0] + wqm[0] + consts
        # -> PE can run K/Q m0-n0 projections and the first scores ~6us in.
        half = 512
        qs_ = [nc.sync, nc.gpsimd, nc.scalar]
        di = 0

        def dma(dst, srcap):
            nonlocal di
            qs_[di % 3].dma_start(dst, srcap)
            di += 1

        for kt_ in range(6):
            dma(xt_tiles[kt_][:, 0:half], xT[kt_ * 128:(kt_ + 1) * 128, 0:half])
        dma(wkm_tiles[0][:], wkm[0])
        dma(wqm_tiles[0][:], wqm[0])
        for kt_ in range(6, 8):
            dma(xt_tiles[kt_][:, 0:half], xT[kt_ * 128:(kt_ + 1) * 128, 0:half])
        # second wave: V weights first (V projections start ~12us in)
        for kt_ in range(8):
            dma(wv_tiles[kt_][:], wv[kt_ * 128:(kt_ + 1) * 128, :])
        dma(bq_sb[:], bq.rearrange("(m p) o -> p (m o)", p=128))
        dma(bk_sb[:], bk.rearrange("(m p) o -> p (m o)", p=128))
        dma(bvb_sb[:], bvb[:])
        for kt_ in range(8):
            dma(xt_tiles[kt_][:, half:2 * half],
                xT[kt_ * 128:(kt_ + 1) * 128, half:2 * half])
        # third wave: remaining W m-groups + xt quarters + wo
        for m in range(1, 4):
            dma(wkm_tiles[m][:], wkm[m])
            dma(wqm_tiles[m][:], wqm[m])
        for kt_ in range(8):
            dma(xt_tiles[kt_][:, 2 * half:3 * half],
                xT[kt_ * 128:(kt_ + 1) * 128, 2 * half:3 * half])
        for dt_ in range(4):
            dma(wo_tiles[dt_][:], wo[dt_ * 128:(dt_ + 1) * 128, :])
        for kt_ in range(8):
            dma(xt_tiles[kt_][:, 3 * half:4 * half],
                xT[kt_ * 128:(kt_ + 1) * 128, 3 * half:4 * half])

        # ones columns of vaug are static: memset each tile to 1.0 once
        for st in range(ST):
            nc.vector.memset(vaug_tiles[st][:], 1.0)

        # ---- PSUM pools ---------------------------------------------------
        # psS(2x2 banks) scores | psC(2x1) AV accum | psB(2x1) projections,
        # later swapped for psO(2x1) + osb staging.
        psS_cm = tc.tile_pool(name="psS", bufs=2, space="PSUM")
        psS = psS_cm.__enter__()
        psC_cm = tc.tile_pool(name="psC", bufs=2, space="PSUM")
        psC = psC_cm.__enter__()
        psB_cm = tc.tile_pool(name="psB", bufs=2, space="PSUM")
        psB = psB_cm.__enter__()

        exp_cm = tc.tile_pool(name="exp", bufs=6)
        exp_pool = exp_cm.__enter__()
        rec_cm = tc.tile_pool(name="rec", bufs=4)
        rec_pool = rec_cm.__enter__()

        state = {"psO": None, "osb": None}

        # ---- building blocks ---------------------------------------------
        def proj_group(wm_tiles, dst, b_sb, m, n):
            p = psB.tile([128, 512], F32, tag="psB", name=f"psb{m}_{n}")
            for kt_ in range(8):
                nc.tensor.matmul(
                    p[:],
                    wm_tiles[m][:, kt_ * 128:(kt_ + 1) * 128],
                    xt_tiles[kt_][:, n * 512:(n + 1) * 512],
                    start=(kt_ == 0), stop=(kt_ == 7),
                )
            nc.vector.tensor_scalar(
                dst[m][:, n * 512:(n + 1) * 512], p[:],
                b_sb[:, m:m + 1], None, op0=mybir.AluOpType.add,
            )

        def v_group(st):
            pv = psB.tile([128, DC], F32, tag="psB", name=f"psv{st}")
            for kt_ in range(8):
                nc.tensor.matmul(
                    pv[:],
                    xt_tiles[kt_][:, st * 128:(st + 1) * 128],
                    wv_tiles[kt_][:],
                    start=(kt_ == 0), stop=(kt_ == 7),
                )
            # bias folded into the evacuation add (bvb is bv broadcast
            # across partitions, sent by the host)
            vt = vaug_tiles[st]
            nc.vector.tensor_add(
                vt[:].rearrange("p (h w) -> p h w", h=H_CORE)[:, :, 0:64],
                pv[:].rearrange("p (h w) -> p h w", h=H_CORE),
                bvb_sb[:].rearrange("p (h w) -> p h w", h=H_CORE),
            )

        def open_psO():
            psB_cm.__exit__(None, None, None)
            psO_cm = tc.tile_pool(name="psO", bufs=2, space="PSUM")
            state["psO"] = (psO_cm, psO_cm.__enter__())
            osb_cm = tc.tile_pool(name="osb", bufs=3)
            state["osb"] = (osb_cm, osb_cm.__enter__())

        def out_group(st, use_scalar=False, copy_eng=None):
            ss = slice(st * 128, (st + 1) * 128)
            psO = state["psO"][1]
            osb_pool = state["osb"][1]
            o_sb = osb_pool.tile([128, D], BF16, tag="osb", name=f"osb{st}")
            # at the tail the Act engine is idle, so its DMA queue helps
            # drain the last 2MB of output faster
            engs = ([nc.sync, nc.gpsimd, nc.scalar, nc.sync] if use_scalar
                    else [nc.sync, nc.gpsimd])
            ce = copy_eng or nc.vector
            for nh in range(2):
                po = psO.tile([128, 512], F32, tag="psO", name=f"po{st}_{nh}")
                for dt_ in range(4):
                    nc.tensor.matmul(
                        po[:],
                        ctx_tiles[dt_][:, ss],
                        wo_tiles[dt_][:, nh * 512:(nh + 1) * 512],
                        start=(dt_ == 0), stop=(dt_ == 3),
                    )
                ce.tensor_copy(o_sb[:, nh * 512:(nh + 1) * 512], po[:])
                if use_scalar:
                    hw = 256
                    for j in range(2):
                        c0 = nh * 512 + j * hw
                        engs[2 * nh + j].dma_start(
                            out[ss, c0:c0 + hw], o_sb[:, c0:c0 + hw])
                else:
                    engs[nh].dma_start(out[ss, nh * 512:(nh + 1) * 512],
                                       o_sb[:, nh * 512:(nh + 1) * 512])

        # ---- filler schedule ---------------------------------------------
        # blk = 4*p_ + q (16 blocks of 8 t2-steps).  Values: list of
        # callables per (blk, t2).
        inject = {}

        def add(blk, t2, fn):
            inject.setdefault((blk, t2), []).append(fn)

        # block 0: V projections (2/step, 2 steps ahead) + K m0 n1..3
        add(0, 0, lambda: v_group(0))
        add(0, 0, lambda: v_group(1))
        add(0, 0, lambda: v_group(2))
        add(0, 0, lambda: v_group(3))
        for t2 in range(1, 7):
            add(0, t2, lambda st=2 * t2 + 2: v_group(st))
            add(0, t2, lambda st=2 * t2 + 3: v_group(st))
        add(0, 1, lambda: proj_group(wkm_tiles, kt_tiles, bk_sb, 0, 1))
        add(0, 3, lambda: proj_group(wkm_tiles, kt_tiles, bk_sb, 0, 2))
        add(0, 5, lambda: proj_group(wkm_tiles, kt_tiles, bk_sb, 0, 3))
        add(0, 7, lambda: proj_group(wqm_tiles, qt_tiles, bq_sb, 0, 1))
        add(1, 0, lambda: proj_group(wqm_tiles, qt_tiles, bq_sb, 0, 2))
        add(1, 2, lambda: proj_group(wqm_tiles, qt_tiles, bq_sb, 0, 3))
        # remaining m1..3 groups, spread so each m-group lands just before
        # its consuming pair: m1 over blks 1-3, m2 over 4-7, m3 over 8-11.
        # This keeps early blocks from crowding out the scores->exp stream.
        rest = []
        for m in range(1, 4):
            for n in range(4):
                rest.append((wkm_tiles, kt_tiles, bk_sb, m, n))
            for n in range(4):
                rest.append((wqm_tiles, qt_tiles, bq_sb, m, n))
        slots = ([(1, t2) for t2 in (2, 4, 6)] +
                 [(2, t2) for t2 in (0, 2, 4, 6)] + [(3, 0)] +
                 [(blk, t2) for blk in range(4, 8) for t2 in (2, 6)] +
                 [(8, 2), (8, 6), (9, 2), (9, 6)] +
                 [(10, 2), (11, 2), (12, 2), (12, 4)])
        for (blk, t2), (wt, dst, bs, m, n) in zip(slots, rest):
            add(blk, t2,
                lambda wt=wt, dst=dst, bs=bs, m=m, n=n:
                    proj_group(wt, dst, bs, m, n))
        add(13, 0, open_psO)
        # output projection: q-block j outputs during block 13+j.  In the
        # last block the PSUM->SBUF copies go to the idle GpSimd engine so
        # the DVE queue is clear for the final normalize chain.
        for j in range(3):
            for i, st in enumerate(range(j * 4, (j + 1) * 4)):
                add(13 + j, 2 * i, lambda st=st: out_group(st))

        # ---- main loop ----------------------------------------------------
        # pre-work: K/Q m0-n0 so the first exp fires as early as possible
        proj_group(wkm_tiles, kt_tiles, bk_sb, 0, 0)
        proj_group(wqm_tiles, qt_tiles, bq_sb, 0, 0)

        for p_ in range(NPAIRS):
            for q in range(QT_):
                blk = 4 * p_ + q
                qs = slice(q * 512, (q + 1) * 512)
                pc0 = psC.tile([128, 512], F32, tag="psC", name=f"pc0_{p_}_{q}")
                pc1 = psC.tile([128, 512], F32, tag="psC", name=f"pc1_{p_}_{q}")
                for kt_ in range(ST):
                    ks = slice(kt_ * 128, (kt_ + 1) * 128)
                    ps = psS.tile([128, 1024], F32, tag="psS",
                                  name=f"ps{p_}_{q}_{kt_}")
                    nc.tensor.matmul(
                        ps[:, 0:512], kt_tiles[p_][0:64, ks],
                        qt_tiles[p_][0:64, qs],
                        start=True, stop=True, tile_position=(0, 0),
                    )
                    nc.tensor.matmul(
                        ps[:, 512:1024], kt_tiles[p_][64:128, ks],
                        qt_tiles[p_][64:128, qs],
                        start=True, stop=True, tile_position=(64, 0),
                    )
                    e = exp_pool.tile([128, 1024], BF16, tag="exp",
                                      name=f"e{p_}_{q}_{kt_}")
                    nc.scalar.activation(e[:], ps[:], AF.Exp, scale=0.125)
                    if kt_ % 2 == 0:
                        for fn in inject.get((blk, kt_ // 2), []):
                            fn()
                    nc.tensor.matmul(
                        pc0[:],
                        vaug_tiles[kt_][:, (2 * p_) * 128:(2 * p_ + 1) * 128],
                        e[:, 0:512],
                        start=(kt_ == 0), stop=(kt_ == ST - 1),
                    )
                    nc.tensor.matmul(
                        pc1[:],
                        vaug_tiles[kt_][:, (2 * p_ + 1) * 128:(2 * p_ + 2) * 128],
                        e[:, 512:1024],
                        start=(kt_ == 0), stop=(kt_ == ST - 1),
                    )

                # normalize: evacuate PSUM (partition-shifted to base 64),
                # then fast-approx reciprocal of the rowsums, then scale.
                cps = []
                for h, pc in ((0, pc0), (1, pc1)):
                    cp = rec_pool.tile([128, 1024], F32, tag="cp",
                                       name=f"cp{p_}_{q}_{h}")
                    nc.vector.tensor_copy(cp[64:128, 0:512], pc[0:64, :])
                    nc.vector.tensor_copy(cp[64:128, 512:1024], pc[64:128, :])
                    cps.append(cp)
                for h, cp in ((0, cps[0]), (1, cps[1])):
                    rec = rec_pool.tile([128, 512], F32, tag="rec",
                                        name=f"rec{p_}_{q}_{h}")
                    # custom-DVE ops require full-partition APs; rows 0:64
                    # of cp are stale garbage and rec[0:64] is never read.
                    nc.vector.reciprocal_approx_fast(rec[:, :],
                                                     cp[:, 512:1024])
                    nc.vector.tensor_mul(
                        ctx_tiles[p_][h * 64:(h + 1) * 64, qs],
                        cp[64:128, 0:512], rec[64:128, :],
                    )

        for st in range((QT_ - 1) * 4, QT_ * 4):
            out_group(st, use_scalar=True)

        state["osb"][0].__exit__(None, None, None)
        state["psO"][0].__exit__(None, None, None)
        rec_cm.__exit__(None, None, None)
        exp_cm.__exit__(None, None, None)
        psC_cm.__exit__(None, None, None)
        psS_cm.__exit__(None, None, None)

    nc.finalize()
    return nc


_CACHED = {}


def _get_graph(S):
    if S not in _CACHED:
        _CACHED[S] = build_graph(S)
    return _CACHED[S]


def make_in_maps(x, Wq, bq, Wk, bk, Wv, bv, Wo, bo):
    bf = ml_dtypes.bfloat16
    in_maps = []
    for c in range(8):
        b, hg = c // 2, c % 2
        sl = slice(512 * hg, 512 * (hg + 1))
        # [m, p, kt*128+c]: W[d=kt*128+p, m*128+c] -> wq_m[m, p, kt, c]
        wq_m = np.ascontiguousarray(
            Wq[:, sl].reshape(8, 128, 4, 128).transpose(2, 1, 0, 3)
            .reshape(4, 128, D)).astype(bf)
        wk_m = np.ascontiguousarray(
            Wk[:, sl].reshape(8, 128, 4, 128).transpose(2, 1, 0, 3)
            .reshape(4, 128, D)).astype(bf)
        in_maps.append({
            "xT": np.ascontiguousarray(x[b].T).astype(bf),
            "wqm": wq_m,
            "wkm": wk_m,
            "wv": np.ascontiguousarray(Wv[:, sl]).astype(bf),
            "wo": np.ascontiguousarray(Wo[sl, :]).astype(bf),
            "bq": np.ascontiguousarray(bq[sl]).reshape(512, 1).astype(np.float32),
            "bk": np.ascontiguousarray(bk[sl]).reshape(512, 1).astype(np.float32),
            "bvb": np.ascontiguousarray(
                np.broadcast_to(bv[sl].reshape(1, 512), (128, 512))).astype(bf),
        })
    return in_maps


def kernel(x, Wq, bq, Wk, bk, Wv, bv, Wo, bo, _trace=False, _tmpdir=None):
    x = np.asarray(x, dtype=np.float32)
    S = x.shape[1]
    nc = _get_graph(S)
    in_maps = make_in_maps(x, np.asarray(Wq), np.asarray(bq), np.asarray(Wk),
                           np.asarray(bk), np.asarray(Wv), np.asarray(bv),
                           np.asarray(Wo), np.asarray(bo))
    res = run_bass_kernel_spmd(
        nc, in_maps, core_ids=list(range(8)), trace=_trace, tmpdir=_tmpdir,
    )
    bo32 = np.asarray(bo, dtype=np.float32)
    outs = [np.asarray(r["out"], dtype=np.float32) for r in res.results]
    full = np.stack([outs[2 * b] + outs[2 * b + 1] + bo32 for b in range(4)])
    kernel.last_results = res
    return full


# revision 34
# speedup vs baseline: 1.1944x; 1.1944x over previous
"""Distributed multi-head attention kernel for one TRN2 chip (8 NeuronCores).

Problem: B=4, S=2048, D=1024, H=16, Dh=64 fp32 attention
    q,k,v = x@W* + b*  (per head)  ->  softmax(q k^T / sqrt(Dh)) v  -> @Wo + bo

Sharding (per the hint): data-parallel over B (4) x tensor-parallel over
head-halves (2) = 8 cores.  Core c = 2*b + hg handles batch b and heads
[8*hg, 8*hg+8) i.e. d_model slice [512*hg, 512*hg+512).  Each core produces
a partial output [2048, 1024] (its 8 heads' contribution through Wo); the
host sums the two partials per batch and adds bo (the unshard step).

Per-core pipeline:
  - features-on-partitions: Q^T/K^T [dc, S] from the QKV matmuls; scores^T
    tiles land [k_seq, q_seq] with k on partitions.
  - softmax: exp() unnormalized on the Act engine; row-sums come from
    ones-columns appended to V (free: matmul cost is free-dim-bound).
  - normalization (1/rowsum) via reciprocal_approx_fast (custom DVE op,
    ~5x faster than InstReciprocal).  NOTE: custom-DVE ops silently
    no-op on partition-offset APs — always issue them on [0:128].
  - schedule: Q/K m0-n0 projections + first scores go FIRST so exp starts
    ~15us in; V projections fill block 0; remaining Q/K m-groups spread
    over blocks 1-12 (each m-group just before its consuming pair) and
    the output projection over blocks 13-15, sized so each step's PE
    work stays near the 2.2us of exp it feeds.  Final 2MB of output DMA
    fans out over 3 queues (Act's queue is idle by then).

Compute dtypes: bf16 matmul operands, fp32 PSUM accumulate, bf16 output
partials (host sums in fp32).  Measured rel-err ~2.7e-3 (gate 2e-2).
fp8 DoubleRow/SwInterleave AV was tried and REVERTED: on real TRN2 those
matmuls run at ~2x the duration of bf16 (no throughput win, cost model
disagrees with HW) and plain-DR needs no layout change but wins nothing.
"""

import sys

sys.path.insert(0, "/opt/trn_rl_repo")

import numpy as np
import ml_dtypes

from contextlib import ExitStack

import concourse.bass as bass
import concourse.tile as tile
from concourse import bacc, mybir
from concourse.bass_utils import run_bass_kernel_spmd

BF16 = mybir.dt.bfloat16
F32 = mybir.dt.float32
FP8 = mybir.dt.float8e4
AF = mybir.ActivationFunctionType
DR = mybir.MatmulPerfMode.DoubleRow


def _install_ntff_hook():
    """Provide antenv.axon_hooks (missing in this image) so that
    run_bass_kernel_spmd(trace=True) can capture NTFF profiles via the
    axon PJRT .so's C ABI."""
    import types, ctypes, contextlib

    if "antenv.axon_hooks" in sys.modules:
        return
    so_path = "/opt/axon/libaxon_pjrt.so"
    mod = types.ModuleType("antenv.axon_hooks")
    _state = {"hook": None}

    def set_axon_ntff_profile_hook(h):
        _state["hook"] = h

    def get_axon_ntff_profile_hook():
        return _state["hook"]

    mod.set_axon_ntff_profile_hook = set_axon_ntff_profile_hook
    mod.get_axon_ntff_profile_hook = get_axon_ntff_profile_hook
    sys.modules["antenv.axon_hooks"] = mod
    import antenv

    antenv.axon_hooks = mod

    try:
        lib = ctypes.CDLL(so_path)
    except OSError:
        return
    if not hasattr(lib, "axon_start_nrt_profile"):
        return
    lib.axon_start_nrt_profile.argtypes = [
        ctypes.POINTER(ctypes.c_int64),
        ctypes.c_size_t,
    ]
    lib.axon_start_nrt_profile.restype = ctypes.c_int64
    lib.axon_stop_nrt_profile.argtypes = [ctypes.c_char_p]
    lib.axon_stop_nrt_profile.restype = ctypes.c_int64

    @contextlib.contextmanager
    def _hook(output_dir, device_ids):
        import jax

        jax.devices()
        if device_ids:
            ids = (ctypes.c_int64 * len(device_ids))(*device_ids)
            rc = lib.axon_start_nrt_profile(ids, len(device_ids))
        else:
            rc = lib.axon_start_nrt_profile(None, 0)
        if rc != 0:
            raise RuntimeError(f"axon_start_nrt_profile rc={rc}")
        try:
            yield
        finally:
            n = lib.axon_stop_nrt_profile(str(output_dir).encode())
            print(f"profile: {n} file(s) written to {output_dir}",
                  file=sys.stderr)

    set_axon_ntff_profile_hook(_hook)


_install_ntff_hook()

D = 1024          # d_model
DC = 512          # per-core d slice (8 heads)
H_CORE = 8        # heads per core
DH = 64           # head dim
NPAIRS = 4        # head pairs per core


def build_graph(S=2048):
    """Build the per-core Bass graph (same graph on all 8 cores)."""
    nc = bacc.Bacc(
        "TRN2",
        target_bir_lowering=False,
        debug=False,
        enable_asserts=False,
        num_devices=8,
    )

    ST = S // 128       # 128-seq tiles (16)
    T2 = ST // 2        # 256-seq k-pair tiles (8)
    QT_ = S // 512      # 512-seq q blocks (4)

    xT = nc.dram_tensor("xT", [D, S], BF16, kind="ExternalInput").ap()
    # wqm/wkm are m-major and pre-transposed to the SBUF tile layout
    # [m-group, p, kt*128+c] so one contiguous DMA brings the whole
    # m-slice needed by a projection group.
    wqm = nc.dram_tensor("wqm", [4, 128, D], BF16, kind="ExternalInput").ap()
    wkm = nc.dram_tensor("wkm", [4, 128, D], BF16, kind="ExternalInput").ap()
    wv = nc.dram_tensor("wv", [D, DC], BF16, kind="ExternalInput").ap()
    wo = nc.dram_tensor("wo", [DC, D], BF16, kind="ExternalInput").ap()
    bq = nc.dram_tensor("bq", [DC, 1], F32, kind="ExternalInput").ap()
    bk = nc.dram_tensor("bk", [DC, 1], F32, kind="ExternalInput").ap()
    bvb = nc.dram_tensor("bvb", [128, DC], BF16, kind="ExternalInput").ap()
    # bf16 output: halves the 8MB/core output DMA; the host sums the two
    # core partials in fp32.  Costs ~2e-3 extra rel-err, well under gate.
    out = nc.dram_tensor("out", [S, D], BF16, kind="ExternalOutput").ap()

    with tile.TileContext(nc) as tc, ExitStack() as ctx:
        # ---- persistent pools --------------------------------------------
        qt_pool = ctx.enter_context(tc.tile_pool(name="qt", bufs=4))
        kt_pool = ctx.enter_context(tc.tile_pool(name="kt", bufs=4))
        vaug_pool = ctx.enter_context(tc.tile_pool(name="vaug", bufs=ST))
        ctx_pool = ctx.enter_context(tc.tile_pool(name="ctxT", bufs=4))
        const_pool = ctx.enter_context(tc.tile_pool(name="consts", bufs=1))
        wo_pool = ctx.enter_context(tc.tile_pool(name="wo", bufs=4))
        xt_pool = ctx.enter_context(tc.tile_pool(name="xt", bufs=8))
        wqk_pool = ctx.enter_context(tc.tile_pool(name="wqk", bufs=8))
        wv_pool = ctx.enter_context(tc.tile_pool(name="wv", bufs=8))

        qt_tiles = [qt_pool.tile([128, S], BF16, tag="qt", name=f"qt{i}")
                    for i in range(4)]
        kt_tiles = [kt_pool.tile([128, S], BF16, tag="kt", name=f"ktt{i}")
                    for i in range(4)]
        # vaug[st]: [128 kpos, 8 heads x (64 V | 64 ones)] bf16
        vaug_tiles = [vaug_pool.tile([128, H_CORE * 128], BF16, tag="vaug",
                                     name=f"vaug{i}") for i in range(ST)]
        ctx_tiles = [ctx_pool.tile([128, S], BF16, tag="ctxT", name=f"ctxT{i}")
                     for i in range(4)]
        xt_tiles = [xt_pool.tile([128, S], BF16, tag="xt", name=f"xtt{i}")
                    for i in range(8)]
        wqm_tiles = [wqk_pool.tile([128, D], BF16, tag="wqk", name=f"wqm{m}")
                     for m in range(4)]
        wkm_tiles = [wqk_pool.tile([128, D], BF16, tag="wqk", name=f"wkm{m}")
                     for m in range(4)]
        wv_tiles = [wv_pool.tile([128, DC], BF16, tag="wv", name=f"wvt{i}")
                    for i in range(8)]
        wo_tiles = [wo_pool.tile([128, D], BF16, tag="wo", name=f"wot{i}")
                    for i in range(4)]

        bvb_sb = const_pool.tile([128, DC], BF16, tag="bvb")
        bq_sb = const_pool.tile([128, DC // 128], F32, tag="bq")
        bk_sb = const_pool.tile([128, DC // 128], F32, tag="bk")

        # ---- DMA plan: 4 queues, critical prefix first -------------------
        # prefix: xt cols 0:512 (all 8 k-tiles) + wkm[0] + wqm[0] + consts
        # -> PE can run K/Q m0-n0 projections and the first scores ~6us in.
        half = 512
        qs_ = [nc.sync, nc.gpsimd, nc.scalar]
        di = 0

        def dma(dst, srcap):
            nonlocal di
            qs_[di % 3].dma_start(dst, srcap)
            di += 1

        for kt_ in range(6):
            dma(xt_tiles[kt_][:, 0:half], xT[kt_ * 128:(kt_ + 1) * 128, 0:half])
        dma(wkm_tiles[0][:], wkm[0])
        dma(wqm_tiles[0][:], wqm[0])
        for kt_ in range(6, 8):
            dma(xt_tiles[kt_][:, 0:half], xT[kt_ * 128:(kt_ + 1) * 128, 0:half])
        # second wave: V weights first (V projections start ~12us in)
        for kt_ in range(8):
            dma(wv_tiles[kt_][:], wv[kt_ * 128:(kt_ + 1) * 128, :])
        dma(bq_sb[:], bq.rearrange("(m p) o -> p (m o)", p=128))
        dma(bk_sb[:], bk.rearrange("(m p) o -> p (m o)", p=128))
        dma(bvb_sb[:], bvb[:])
        for kt_ in range(8):
            dma(xt_tiles[kt_][:, half:2 * half],
                xT[kt_ * 128:(kt_ + 1) * 128, half:2 * half])
        # third wave: remaining W m-groups + xt quarters + wo
        for m in range(1, 4):
            dma(wkm_tiles[m][:], wkm[m])
            dma(wqm_tiles[m][:], wqm[m])
        for kt_ in range(8):
            dma(xt_tiles[kt_][:, 2 * half:3 * half],
                xT[kt_ * 128:(kt_ + 1) * 128, 2 * half:3 * half])
        for dt_ in range(4):
            dma(wo_tiles[dt_][:], wo[dt_ * 128:(dt_ + 1) * 128, :])
        for kt_ in range(8):
            dma(xt_tiles[kt_][:, 3 * half:4 * half],
                xT[kt_ * 128:(kt_ + 1) * 128, 3 * half:4 * half])

        # ones columns of vaug are static: memset each tile to 1.0 once
        for st in range(ST):
            nc.vector.memset(vaug_tiles[st][:], 1.0)

        # ---- PSUM pools ---------------------------------------------------
        # psS(2x2 banks) scores | psC(2x1) AV accum | psB(2x1) projections,
        # later swapped for psO(2x1) + osb staging.
        psS_cm = tc.tile_pool(name="psS", bufs=2, space="PSUM")
        psS = psS_cm.__enter__()
        psC_cm = tc.tile_pool(name="psC", bufs=2, space="PSUM")
        psC = psC_cm.__enter__()
        psB_cm = tc.tile_pool(name="psB", bufs=2, space="PSUM")
        psB = psB_cm.__enter__()

        exp_cm = tc.tile_pool(name="exp", bufs=6)
        exp_pool = exp_cm.__enter__()
        rec_cm = tc.tile_pool(name="rec", bufs=4)
        rec_pool = rec_cm.__enter__()

        state = {"psO": None, "osb": None}

        # ---- building blocks ---------------------------------------------
        def proj_group(wm_tiles, dst, b_sb, m, n):
            p = psB.tile([128, 512], F32, tag="psB", name=f"psb{m}_{n}")
            for kt_ in range(8):
                nc.tensor.matmul(
                    p[:],
                    wm_tiles[m][:, kt_ * 128:(kt_ + 1) * 128],
                    xt_tiles[kt_][:, n * 512:(n + 1) * 512],
                    start=(kt_ == 0), stop=(kt_ == 7),
                )
            nc.vector.tensor_scalar(
                dst[m][:, n * 512:(n + 1) * 512], p[:],
                b_sb[:, m:m + 1], None, op0=mybir.AluOpType.add,
            )

        def v_group(st):
            pv = psB.tile([128, DC], F32, tag="psB", name=f"psv{st}")
            for kt_ in range(8):
                nc.tensor.matmul(
                    pv[:],
                    xt_tiles[kt_][:, st * 128:(st + 1) * 128],
                    wv_tiles[kt_][:],
                    start=(kt_ == 0), stop=(kt_ == 7),
                )
            # bias folded into the evacuation add (bvb is bv broadcast
            # across partitions, sent by the host)
            vt = vaug_tiles[st]
            nc.vector.tensor_add(
                vt[:].rearrange("p (h w) -> p h w", h=H_CORE)[:, :, 0:64],
                pv[:].rearrange("p (h w) -> p h w", h=H_CORE),
                bvb_sb[:].rearrange("p (h w) -> p h w", h=H_CORE),
            )

        def open_psO():
            psB_cm.__exit__(None, None, None)
            psO_cm = tc.tile_pool(name="psO", bufs=2, space="PSUM")
            state["psO"] = (psO_cm, psO_cm.__enter__())
            osb_cm = tc.tile_pool(name="osb", bufs=3)
            state["osb"] = (osb_cm, osb_cm.__enter__())

        def out_group(st, use_scalar=False, copy_eng=None):
            ss = slice(st * 128, (st + 1) * 128)
            psO = state["psO"][1]
            osb_pool = state["osb"][1]
            o_sb = osb_pool.tile([128, D], BF16, tag="osb", name=f"osb{st}")
            # at the tail the Act engine is idle, so its DMA queue helps
            # drain the last 2MB of output faster
            engs = ([nc.sync, nc.gpsimd, nc.scalar, nc.sync] if use_scalar
                    else [nc.sync, nc.gpsimd])
            ce = copy_eng or nc.vector
            for nh in range(2):
                po = psO.tile([128, 512], F32, tag="psO", name=f"po{st}_{nh}")
                for dt_ in range(4):
                    nc.tensor.matmul(
                        po[:],
                        ctx_tiles[dt_][:, ss],
                        wo_tiles[dt_][:, nh * 512:(nh + 1) * 512],
                        start=(dt_ == 0), stop=(dt_ == 3),
                    )
                ce.tensor_copy(o_sb[:, nh * 512:(nh + 1) * 512], po[:])
                if use_scalar:
                    hw = 256
                    for j in range(2):
                        c0 = nh * 512 + j * hw
                        engs[2 * nh + j].dma_start(
                            out[ss, c0:c0 + hw], o_sb[:, c0:c0 + hw])
                else:
                    engs[nh].dma_start(out[ss, nh * 512:(nh + 1) * 512],
                                       o_sb[:, nh * 512:(nh + 1) * 512])

        # ---- filler schedule ---------------------------------------------
        # blk = 4*p_ + q (16 blocks of 8 t2-steps).  Values: list of
        # callables per (blk, t2).
        inject = {}

        def add(blk, t2, fn):
            inject.setdefault((blk, t2), []).append(fn)

        # block 0: V projections (2/step, 2 steps ahead) + K m0 n1..3
        add(0, 0, lambda: v_group(0))
        add(0, 0, lambda: v_group(1))
        add(0, 0, lambda: v_group(2))
        add(0, 0, lambda: v_group(3))
        for t2 in range(1, 7):
            add(0, t2, lambda st=2 * t2 + 2: v_group(st))
            add(0, t2, lambda st=2 * t2 + 3: v_group(st))
        add(0, 1, lambda: proj_group(wkm_tiles, kt_tiles, bk_sb, 0, 1))
        add(0, 3, lambda: proj_group(wkm_tiles, kt_tiles, bk_sb, 0, 2))
        add(0, 5, lambda: proj_group(wkm_tiles, kt_tiles, bk_sb, 0, 3))
        add(0, 7, lambda: proj_group(wqm_tiles, qt_tiles, bq_sb, 0, 1))
        add(1, 0, lambda: proj_group(wqm_tiles, qt_tiles, bq_sb, 0, 2))
        add(1, 2, lambda: proj_group(wqm_tiles, qt_tiles, bq_sb, 0, 3))
        # remaining m1..3 groups, spread so each m-group lands just before
        # its consuming pair: m1 over blks 1-3, m2 over 4-7, m3 over 8-11.
        # This keeps early blocks from crowding out the scores->exp stream.
        rest = []
        for m in range(1, 4):
            for n in range(4):
                rest.append((wkm_tiles, kt_tiles, bk_sb, m, n))
            for n in range(4):
                rest.append((wqm_tiles, qt_tiles, bq_sb, m, n))
        slots = ([(1, t2) for t2 in (2, 4, 6)] +
                 [(2, t2) for t2 in (0, 2, 4, 6)] + [(3, 0)] +
                 [(blk, t2) for blk in range(4, 8) for t2 in (2, 6)] +
                 [(8, 2), (8, 6), (9, 2), (9, 6)] +
                 [(10, 2), (11, 2), (12, 2), (12, 4)])
        for (blk, t2), (wt, dst, bs, m, n) in zip(slots, rest):
            add(blk, t2,
                lambda wt=wt, dst=dst, bs=bs, m=m, n=n:
                    proj_group(wt, dst, bs, m, n))
        add(13, 0, open_psO)
        # output projection: q-block j outputs during block 13+j.  In the
        # last block the PSUM->SBUF copies go to the idle GpSimd engine so
        # the DVE queue is clear for the final normalize chain.
        for j in range(3):
            for i, st in enumerate(range(j * 4, (j + 1) * 4)):
                add(13 + j, 2 * i, lambda st=st: out_group(st))

        # ---- main loop ----------------------------------------------------
        # pre-work: K/Q m0-n0 so the first exp fires as early as possible
        proj_group(wkm_tiles, kt_tiles, bk_sb, 0, 0)
        proj_group(wqm_tiles, qt_tiles, bq_sb, 0, 0)

        for p_ in range(NPAIRS):
            for q in range(QT_):
                blk = 4 * p_ + q
                qs = slice(q * 512, (q + 1) * 512)
                pc0 = psC.tile([128, 512], F32, tag="psC", name=f"pc0_{p_}_{q}")
                pc1 = psC.tile([128, 512], F32, tag="psC", name=f"pc1_{p_}_{q}")
                for kt_ in range(ST):
                    ks = slice(kt_ * 128, (kt_ + 1) * 128)
                    ps = psS.tile([128, 1024], F32, tag="psS",
                                  name=f"ps{p_}_{q}_{kt_}")
                    nc.tensor.matmul(
                        ps[:, 0:512], kt_tiles[p_][0:64, ks],
                        qt_tiles[p_][0:64, qs],
                        start=True, stop=True, tile_position=(0, 0),
                    )
                    nc.tensor.matmul(
                        ps[:, 512:1024], kt_tiles[p_][64:128, ks],
                        qt_tiles[p_][64:128, qs],
                        start=True, stop=True, tile_position=(64, 0),
                    )
                    e = exp_pool.tile([128, 1024], BF16, tag="exp",
                                      name=f"e{p_}_{q}_{kt_}")
                    nc.scalar.activation(e[:], ps[:], AF.Exp, scale=0.125)
                    if kt_ % 2 == 0:
                        for fn in inject.get((blk, kt_ // 2), []):
                            fn()
                    nc.tensor.matmul(
                        pc0[:],
                        vaug_tiles[kt_][:, (2 * p_) * 128:(2 * p_ + 1) * 128],
                        e[:, 0:512],
                        start=(kt_ == 0), stop=(kt_ == ST - 1),
                    )
                    nc.tensor.matmul(
                        pc1[:],
                        vaug_tiles[kt_][:, (2 * p_ + 1) * 128:(2 * p_ + 2) * 128],
                        e[:, 512:1024],
                        start=(kt_ == 0), stop=(kt_ == ST - 1),
                    )

                # normalize: evacuate PSUM (partition-shifted to base 64),
                # then fast-approx reciprocal of the rowsums, then scale.
                last = (p_ == NPAIRS - 1 and q == QT_ - 1)
                if not last:
                    cps = []
                    for h, pc in ((0, pc0), (1, pc1)):
                        cp = rec_pool.tile([128, 1024], F32, tag="cp",
                                           name=f"cp{p_}_{q}_{h}")
                        nc.vector.tensor_copy(cp[64:128, 0:512], pc[0:64, :])
                        nc.vector.tensor_copy(cp[64:128, 512:1024],
                                              pc[64:128, :])
                        cps.append(cp)
                    for h, cp in ((0, cps[0]), (1, cps[1])):
                        rec = rec_pool.tile([128, 512], F32, tag="rec",
                                            name=f"rec{p_}_{q}_{h}")
                        # custom-DVE ops require full-partition APs; rows
                        # 0:64 of cp are stale garbage; rec[0:64] unused.
                        nc.vector.reciprocal_approx_fast(rec[:, :],
                                                         cp[:, 512:1024])
                        nc.vector.tensor_mul(
                            ctx_tiles[p_][h * 64:(h + 1) * 64, qs],
                            cp[64:128, 0:512], rec[64:128, :],
                        )
                else:
                    # last block: normalize in 256-col chunks and start the
                    # q3 output projections as soon as their columns are
                    # ready, pipelining the tail DVE chain with the PE.
                    for ci in range(2):
                        cs = slice(q * 512 + ci * 256, q * 512 + ci * 256 + 256)
                        pcs_ = slice(ci * 256, ci * 256 + 256)
                        for h, pc in ((0, pc0), (1, pc1)):
                            cp = rec_pool.tile([128, 512], F32, tag="cp",
                                               name=f"cpL{ci}_{h}")
                            nc.vector.tensor_copy(cp[64:128, 0:256],
                                                  pc[0:64, pcs_])
                            nc.vector.tensor_copy(cp[64:128, 256:512],
                                                  pc[64:128, pcs_])
                            rec = rec_pool.tile([128, 256], F32, tag="rec",
                                                name=f"recL{ci}_{h}")
                            nc.vector.reciprocal_approx_fast(rec[:, :],
                                                             cp[:, 256:512])
                            nc.vector.tensor_mul(
                                ctx_tiles[p_][h * 64:(h + 1) * 64, cs],
                                cp[64:128, 0:256], rec[64:128, :],
                            )
                        out_group(12 + 2 * ci, use_scalar=True)
                        out_group(13 + 2 * ci, use_scalar=True)

        state["osb"][0].__exit__(None, None, None)
        state["psO"][0].__exit__(None, None, None)
        rec_cm.__exit__(None, None, None)
        exp_cm.__exit__(None, None, None)
        psC_cm.__exit__(None, None, None)
        psS_cm.__exit__(None, None, None)

    nc.finalize()
    return nc


_CACHED = {}


def _get_graph(S):
    if S not in _CACHED:
        _CACHED[S] = build_graph(S)
    return _CACHED[S]


def make_in_maps(x, Wq, bq, Wk, bk, Wv, bv, Wo, bo):
    bf = ml_dtypes.bfloat16
    in_maps = []
    for c in range(8):
        b, hg = c // 2, c % 2
        sl = slice(512 * hg, 512 * (hg + 1))
        # [m, p, kt*128+c]: W[d=kt*128+p, m*128+c] -> wq_m[m, p, kt, c]
        wq_m = np.ascontiguousarray(
            Wq[:, sl].reshape(8, 128, 4, 128).transpose(2, 1, 0, 3)
            .reshape(4, 128, D)).astype(bf)
        wk_m = np.ascontiguousarray(
            Wk[:, sl].reshape(8, 128, 4, 128).transpose(2, 1, 0, 3)
            .reshape(4, 128, D)).astype(bf)
        in_maps.append({
            "xT": np.ascontiguousarray(x[b].T).astype(bf),
            "wqm": wq_m,
            "wkm": wk_m,
            "wv": np.ascontiguousarray(Wv[:, sl]).astype(bf),
            "wo": np.ascontiguousarray(Wo[sl, :]).astype(bf),
            "bq": np.ascontiguousarray(bq[sl]).reshape(512, 1).astype(np.float32),
            "bk": np.ascontiguousarray(bk[sl]).reshape(512, 1).astype(np.float32),
            "bvb": np.ascontiguousarray(
                np.broadcast_to(bv[sl].reshape(1, 512), (128, 512))).astype(bf),
        })
    return in_maps


def kernel(x, Wq, bq, Wk, bk, Wv, bv, Wo, bo, _trace=False, _tmpdir=None):
    x = np.asarray(x, dtype=np.float32)
    S = x.shape[1]
    nc = _get_graph(S)
    in_maps = make_in_maps(x, np.asarray(Wq), np.asarray(bq), np.asarray(Wk),
                           np.asarray(bk), np.asarray(Wv), np.asarray(bv),
                           np.asarray(Wo), np.asarray(bo))
    res = run_bass_kernel_spmd(
        nc, in_maps, core_ids=list(range(8)), trace=_trace, tmpdir=_tmpdir,
    )
    bo32 = np.asarray(bo, dtype=np.float32)
    outs = [np.asarray(r["out"], dtype=np.float32) for r in res.results]
    full = np.stack([outs[2 * b] + outs[2 * b + 1] + bo32 for b in range(4)])
    kernel.last_results = res
    return full
